# revision 9
# baseline (speedup 1.0000x reference)
"""Trainium2 Bass kernel for nn_CalibrationNetwork (MoE-routed 3-layer MLP + softmax).

Strategy
--------
Host side (numpy): sort samples by judge id, pad each judge group to a
multiple of 256 samples ("supertile"), distribute supertiles round-robin
over 8 cores (20 slots/core covers the worst case sum_j ceil(c_j/256) <= 160).
Per supertile the host emits:
  - xt [12, 1024] bf16: transposed x with bias-ones rows (rows s*6+d; d=0 is
    the ones row), question-pair p in columns p*256..(p+1)*256.
  - wa [12, 552] bf16: L1 stationary blocks (block-diag over the question
    pair) rows s*6+d, cols p*128 + s*64+h; partition 0 cols 512..552 holds
    the L3 bias row b3 laid out as (p,s,o).
  - wb [128, 172] bf16: cols 0..128 L2 block-diag stationary [[W2,0],[0,W2]];
    cols 128..168 L3 moving operand (V block per pair, cols p*10+s*5+o);
    cols 168..170 the L2 bias column as raw f32 bits (bitcast on device).
All judge-specific tables are combined with the shared ones on the host
(W1+W1_a[j] etc.), so the device never gathers.

Device (per supertile, all engines pipelined by Tile):
  L1: 4 matmuls  psum1[(s,h1), (p,b)] += wa_p^T @ xt_p          (K=12, M=128, N=256)
  relu1 (DVE):   z1 = max(psum1, 0) -> bf16                     (bias via ones row)
  L2: 4 matmuls  psum2[(s,h2), (p,b)] += wb[:,:128]^T @ z1_p    (K=128, M=128, N=256)
  relu2 (ACT):   z2 = relu(psum2 + b2)  -> bf16                 (per-partition bias)
  L3: 2 bias matmuls (ones_col^T @ b3row broadcasts b3 over samples)
      + 8 matmuls psum3[b, (blk,p,s,o)] += z2_slice^T @ vb_p    (K=128, M=128, N=10)
  softmax: exp (ACT), grouped reduce_sum + reciprocal + multiply (DVE)
  DMA out [2,128,40] f32 per supertile; host strips padding and unsorts.
"""

import numpy as np
import ml_dtypes

B, J, Q, O = 32768, 32, 7, 5
H = 64            # H1 == H2
ST = 256          # samples per supertile
T = 20            # supertiles per core (worst case 32768/256 + 32 = 160 = 8*20)
N_CORES = 8
QP = 4            # question pairs (Q=7 padded to 8)

_bf16 = ml_dtypes.bfloat16
_cache = {}


# ----------------------------------------------------------------------------
# device program
# ----------------------------------------------------------------------------

def _build_program():
    import concourse.bacc as bacc
    import concourse.tile as tile
    import concourse.mybir as mybir
    import concourse.bass as bass
    from contextlib import ExitStack

    bf = mybir.dt.bfloat16
    f32 = mybir.dt.float32
    AF = mybir.ActivationFunctionType

    nc = bacc.Bacc("TRN2", target_bir_lowering=False, debug=False)
    xt_d = nc.dram_tensor("xt", (T, 12, QP * ST), bf, kind="ExternalInput")
    wa_d = nc.dram_tensor("wa", (T, 12, 592), bf, kind="ExternalInput")
    wb_d = nc.dram_tensor("wb", (T, 128, 172), bf, kind="ExternalInput")
    out_d = nc.dram_tensor("out", (T, 2, 128, 40), f32, kind="ExternalOutput")

    def bcast_last(ap, n):
        return bass.AP(ap.tensor, ap.offset, list(ap.ap) + [[0, n]])

    with ExitStack() as ctx:
        tc = ctx.enter_context(tile.TileContext(nc))
        cpool = ctx.enter_context(tc.tile_pool(name="const", bufs=1))
        inp = ctx.enter_context(tc.tile_pool(name="inp", bufs=3))
        zpool = ctx.enter_context(tc.tile_pool(name="z", bufs=2))
        spool = ctx.enter_context(tc.tile_pool(name="soft", bufs=2))
        pp1 = ctx.enter_context(tc.tile_pool(name="pp1", bufs=2, space="PSUM"))
        pp2 = ctx.enter_context(tc.tile_pool(name="pp2", bufs=1, space="PSUM"))
        pp3 = ctx.enter_context(tc.tile_pool(name="pp3", bufs=2, space="PSUM"))

        ones_col = cpool.tile([1, 128], bf)
        nc.vector.memset(ones_col[:], 1.0)

        for t in range(T):
            xt = inp.tile([12, QP * ST], bf, tag="xt")
            nc.sync.dma_start(xt[:], xt_d.ap()[t])
            wa = inp.tile([12, 592], bf, tag="wa")
            nc.sync.dma_start(wa[:], wa_d.ap()[t])
            wb = inp.tile([128, 172], bf, tag="wb")
            nc.sync.dma_start(wb[:], wb_d.ap()[t])

            # L1
            p1 = pp1.tile([128, QP * ST], f32, tag="p1")
            for p in range(QP):
                nc.tensor.matmul(
                    p1[:, p * ST:(p + 1) * ST],
                    wa[:, p * 128:(p + 1) * 128],
                    xt[:, p * ST:(p + 1) * ST],
                    start=True, stop=True)
            z1 = zpool.tile([128, QP * ST], bf, tag="z1")
            nc.vector.tensor_scalar_max(z1[:], p1[:], 0.0)

            # L2
            p2 = pp2.tile([128, QP * ST], f32, tag="p2")
            for p in range(QP):
                nc.tensor.matmul(
                    p2[:, p * ST:(p + 1) * ST],
                    wb[:, 0:128],
                    z1[:, p * ST:(p + 1) * ST],
                    start=True, stop=True)
            z2 = zpool.tile([128, QP * ST], bf, tag="z2")
            b2ap = wb[:, 168:170].bitcast(f32)
            nc.scalar.activation(z2[:], p2[:], AF.Relu, bias=b2ap, scale=1.0)

            # L3: bias broadcast then per-(pair, block) products
            p3 = pp3.tile([128, 80], f32, tag="p3")
            nc.tensor.matmul(
                p3[:, 0:80], ones_col[:], wa[0:1, 512:592],
                start=True, stop=False)
            for i, (p, blk) in enumerate([(p, blk) for p in range(QP)
                                          for blk in range(2)]):
                nc.tensor.matmul(
                    p3[:, blk * 40 + p * 10: blk * 40 + p * 10 + 10],
                    z2[:, p * ST + blk * 128: p * ST + blk * 128 + 128],
                    wb[:, 128 + p * 10: 128 + p * 10 + 10],
                    start=False, stop=(i == QP * 2 - 1))

            # softmax over each group of 5
            exps = spool.tile([128, 80], f32, tag="exps")
            nc.scalar.activation(exps[:], p3[:], AF.Exp)
            den = spool.tile([128, 16], f32, tag="den")
            nc.vector.reduce_sum(
                den[:], exps[:].rearrange("p (g o) -> p g o", o=5),
                axis=mybir.AxisListType.X)
            rden = spool.tile([128, 16], f32, tag="rden")
            nc.vector.reciprocal_approx_fast(rden[:], den[:])
            probs = spool.tile([128, 80], f32, tag="probs")
            nc.vector.tensor_tensor(
                probs[:].rearrange("p (g o) -> p g o", o=5),
                exps[:].rearrange("p (g o) -> p g o", o=5),
                bcast_last(rden[:], 5),
                mybir.AluOpType.mult)

            nc.scalar.dma_start(
                out_d.ap()[t].rearrange("k p c -> p k c"),
                probs[:].rearrange("p (k c) -> p k c", k=2))

    nc.compile()
    return nc


def _get_program():
    if "nc" not in _cache:
        _cache["nc"] = _build_program()
    return _cache["nc"]


# ----------------------------------------------------------------------------
# host-side data prep
# ----------------------------------------------------------------------------

def _expert_blobs(W1, W1_a, W2, W2_a, V, V_a):
    """Per-expert wa [J,64,128] and wb [J,128,172] arrays (uint16 bf16 bits)."""
    W1c = (W1[None] + W1_a).astype(np.float32)    # [J,Q,H,O+1]
    W2c = (W2[None] + W2_a).astype(np.float32)    # [J,H,H+1]
    Vc = (V[None] + V_a).astype(np.float32)       # [J,Q,O,H+1]

    wa = np.zeros((J, 12, 592), np.float32)
    for q in range(Q):
        p, s = q // 2, q % 2
        # [J, d, h] <- W1c[:, q] is [J, h, d]
        wa[:, s * 6:s * 6 + 6, p * 128 + s * 64: p * 128 + (s + 1) * 64] = \
            W1c[:, q].transpose(0, 2, 1)
        for blk in range(2):
            wa[:, 0, 512 + blk * 40 + p * 10 + s * 5:
               512 + blk * 40 + p * 10 + s * 5 + 5] = Vc[:, q, :, 0]
    wa16 = wa.astype(_bf16).view(np.uint16)

    wb = np.zeros((J, 128, 168), np.float32)
    w2w = W2c[:, :, 1:].transpose(0, 2, 1)        # [J, i, h2]
    for s in range(2):
        wb[:, s * 64:(s + 1) * 64, s * 64:(s + 1) * 64] = w2w
    for q in range(Q):
        p, s = q // 2, q % 2
        # [J, h2, o] <- Vc[:, q, :, 1:] is [J, o, h2]
        wb[:, s * 64:(s + 1) * 64, 128 + p * 10 + s * 5: 128 + p * 10 + s * 5 + 5] = \
            Vc[:, q, :, 1:].transpose(0, 2, 1)
    wb16 = np.zeros((J, 128, 172), np.uint16)
    wb16[:, :, :168] = wb.astype(_bf16).view(np.uint16)
    b2 = np.concatenate([W2c[:, :, 0], W2c[:, :, 0]], axis=1)  # [J, 128]
    wb16[:, :, 168:170] = b2.astype(np.float32).view(np.uint16).reshape(J, 128, 2)
    return wa16, wb16


def _plan(judge_ids):
    """Supertile schedule: list of (judge, sample_idx_array), core/slot map."""
    jid = np.asarray(judge_ids).astype(np.int64).ravel()
    assert jid.shape[0] == B
    order = np.argsort(jid, kind="stable")
    counts = np.bincount(jid, minlength=J)
    tiles = []
    pos = 0
    for j in range(J):
        g = order[pos:pos + counts[j]]
        pos += counts[j]
        for s in range(0, len(g), ST):
            tiles.append((j, g[s:s + ST]))
    assert len(tiles) <= N_CORES * T, f"{len(tiles)} supertiles > capacity"
    return tiles


def _prepare_inputs(x, judge_ids, W1, W1_a, W2, W2_a, V, V_a):
    x = np.ascontiguousarray(np.asarray(x, dtype=np.float32))
    wa16, wb16 = _expert_blobs(*(np.asarray(a, dtype=np.float32)
                                 for a in (W1, W1_a, W2, W2_a, V, V_a)))
    tiles = _plan(judge_ids)

    judge_mat = np.zeros((N_CORES, T), np.int64)        # expert per slot
    xg = np.zeros((N_CORES, T, ST, Q, O), np.float32)   # gathered x
    for i, (j, g) in enumerate(tiles):
        k, t = i % N_CORES, i // N_CORES
        judge_mat[k, t] = j
        xg[k, t, :len(g)] = x[g]

    xt = np.zeros((N_CORES, T, 12, QP * ST), np.float32)
    xt[:, :, 0, :] = 1.0
    xt[:, :, 6, :] = 1.0   # dummy q=7 gets ones too; its weights are zero
    for q in range(Q):
        p, s = q // 2, q % 2
        xt[:, :, s * 6 + 1:s * 6 + 6, p * ST:(p + 1) * ST] = \
            xg[:, :, :, q, :].transpose(0, 1, 3, 2)
    xt16 = xt.astype(_bf16).view(np.uint16)

    in_maps = []
    for k in range(N_CORES):
        in_maps.append({
            "xt": np.ascontiguousarray(xt16[k]).view(_bf16),
            "wa": np.ascontiguousarray(wa16[judge_mat[k]]).view(_bf16),
            "wb": np.ascontiguousarray(wb16[judge_mat[k]]).view(_bf16),
        })
    return in_maps, tiles


def _assemble_output(results, tiles):
    out = np.empty((B, Q, O), np.float32)
    for i, (_, g) in enumerate(tiles):
        k, t = i % N_CORES, i // N_CORES
        rows = results[k]["out"][t].reshape(ST, 40)[:len(g), :35]
        out[g] = rows.reshape(len(g), Q, O)
    return out


# ----------------------------------------------------------------------------
# entry point
# ----------------------------------------------------------------------------

def kernel(x, judge_ids, W1, W1_a, W2, W2_a, V, V_a):
    from concourse import bass_utils
    nc = _get_program()
    in_maps, tiles = _prepare_inputs(x, judge_ids, W1, W1_a, W2, W2_a, V, V_a)
    res = bass_utils.run_bass_kernel_spmd(
        nc, in_maps, core_ids=list(range(N_CORES)), trace=False)
    return _assemble_output(res.results, tiles)


# expose for test harness reuse
def run_with_results(x, judge_ids, W1, W1_a, W2, W2_a, V, V_a, trace=False,
                     **kwargs):
    from concourse import bass_utils
    nc = _get_program()
    in_maps, tiles = _prepare_inputs(x, judge_ids, W1, W1_a, W2, W2_a, V, V_a)
    res = bass_utils.run_bass_kernel_spmd(
        nc, in_maps, core_ids=list(range(N_CORES)), trace=trace, **kwargs)
    return _assemble_output(res.results, tiles), res


# revision 15
# speedup vs baseline: 1.0012x; 1.0012x over previous
"""Trainium2 Bass kernel for nn_CalibrationNetwork (MoE-routed 3-layer MLP + softmax).

Strategy
--------
Host side (numpy): sort samples by judge id, pad each judge group to a
multiple of 256 samples ("supertile"), distribute supertiles round-robin
over 8 cores (20 slots/core covers the worst case sum_j ceil(c_j/256) <= 160).
Per supertile the host emits:
  - xt [12, 1024] bf16: transposed x with bias-ones rows (rows s*6+d; d=0 is
    the ones row), question-pair p in columns p*256..(p+1)*256.
  - wa [12, 552] bf16: L1 stationary blocks (block-diag over the question
    pair) rows s*6+d, cols p*128 + s*64+h; partition 0 cols 512..552 holds
    the L3 bias row b3 laid out as (p,s,o).
  - wb [128, 172] bf16: cols 0..128 L2 block-diag stationary [[W2,0],[0,W2]];
    cols 128..168 L3 moving operand (V block per pair, cols p*10+s*5+o);
    cols 168..170 the L2 bias column as raw f32 bits (bitcast on device).
All judge-specific tables are combined with the shared ones on the host
(W1+W1_a[j] etc.), so the device never gathers.

Device (per supertile, all engines pipelined by Tile):
  L1: 4 matmuls  psum1[(s,h1), (p,b)] += wa_p^T @ xt_p          (K=12, M=128, N=256)
  relu1 (DVE):   z1 = max(psum1, 0) -> bf16                     (bias via ones row)
  L2: 4 matmuls  psum2[(s,h2), (p,b)] += wb[:,:128]^T @ z1_p    (K=128, M=128, N=256)
  relu2 (ACT):   z2 = relu(psum2 + b2)  -> bf16                 (per-partition bias)
  L3: 2 bias matmuls (ones_col^T @ b3row broadcasts b3 over samples)
      + 8 matmuls psum3[b, (blk,p,s,o)] += z2_slice^T @ vb_p    (K=128, M=128, N=10)
  softmax: exp (ACT), grouped reduce_sum + reciprocal + multiply (DVE)
  DMA out [2,128,40] f32 per supertile; host strips padding and unsorts.
"""

import numpy as np
import ml_dtypes

B, J, Q, O = 32768, 32, 7, 5
H = 64            # H1 == H2
ST = 256          # samples per supertile
T = 20            # supertiles per core (worst case 32768/256 + 32 = 160 = 8*20)
N_CORES = 8
QP = 4            # question pairs (Q=7 padded to 8)
XW = QP * ST + 592  # merged xt+wa row length

_bf16 = ml_dtypes.bfloat16
_cache = {}


# ----------------------------------------------------------------------------
# device program
# ----------------------------------------------------------------------------

def _build_program():
    import concourse.bacc as bacc
    import concourse.tile as tile
    import concourse.mybir as mybir
    import concourse.bass as bass
    from contextlib import ExitStack

    bf = mybir.dt.bfloat16
    f32 = mybir.dt.float32
    AF = mybir.ActivationFunctionType

    nc = bacc.Bacc("TRN2", target_bir_lowering=False, debug=False)
    # xw = xt (cols 0..1024) ++ wa (cols 1024..1616, b3 at 1024+512..+592)
    xw_d = nc.dram_tensor("xw", (T, 12, XW), bf, kind="ExternalInput")
    wb_d = nc.dram_tensor("wb", (T, 128, 172), bf, kind="ExternalInput")
    out_d = nc.dram_tensor("out", (T, 128, 80), f32, kind="ExternalOutput")

    def bcast_last(ap, n):
        return bass.AP(ap.tensor, ap.offset, list(ap.ap) + [[0, n]])

    with ExitStack() as ctx:
        tc = ctx.enter_context(tile.TileContext(nc))
        cpool = ctx.enter_context(tc.tile_pool(name="const", bufs=1))
        inp = ctx.enter_context(tc.tile_pool(name="inp", bufs=3))
        zpool = ctx.enter_context(tc.tile_pool(name="z", bufs=2))
        spool = ctx.enter_context(tc.tile_pool(name="soft", bufs=2))
        pp1 = ctx.enter_context(tc.tile_pool(name="pp1", bufs=2, space="PSUM"))
        pp2 = ctx.enter_context(tc.tile_pool(name="pp2", bufs=1, space="PSUM"))
        pp3 = ctx.enter_context(tc.tile_pool(name="pp3", bufs=2, space="PSUM"))

        ones_col = cpool.tile([1, 128], bf)
        nc.vector.memset(ones_col[:], 1.0)

        # two supertiles per macro-iteration: DMAs and softmax tail batched
        for m in range(T // 2):
            xw = inp.tile([12, 2 * XW], bf, tag="xw")
            nc.sync.dma_start(
                xw[:].rearrange("p (t c) -> p t c", t=2),
                xw_d.ap()[2 * m:2 * m + 2].rearrange("t p c -> p t c"))
            wb = inp.tile([128, 2 * 172], bf, tag="wb")
            nc.sync.dma_start(
                wb[:].rearrange("p (t c) -> p t c", t=2),
                wb_d.ap()[2 * m:2 * m + 2].rearrange("t p c -> p t c"))

            exps = spool.tile([128, 160], f32, tag="exps")
            for u in range(2):
                xo, wo = u * XW, u * XW + QP * ST   # xt / wa col offsets
                wbo = u * 172
                # L1
                p1 = pp1.tile([128, QP * ST], f32, tag="p1")
                for p in range(QP):
                    nc.tensor.matmul(
                        p1[:, p * ST:(p + 1) * ST],
                        xw[:, wo + p * 128: wo + (p + 1) * 128],
                        xw[:, xo + p * ST: xo + (p + 1) * ST],
                        start=True, stop=True)
                z1 = zpool.tile([128, QP * ST], bf, tag="z1")
                nc.vector.tensor_scalar_max(z1[:], p1[:], 0.0)

                # L2
                p2 = pp2.tile([128, QP * ST], f32, tag="p2")
                for p in range(QP):
                    nc.tensor.matmul(
                        p2[:, p * ST:(p + 1) * ST],
                        wb[:, wbo:wbo + 128],
                        z1[:, p * ST:(p + 1) * ST],
                        start=True, stop=True)
                z2 = zpool.tile([128, QP * ST], bf, tag="z2")
                b2ap = wb[:, wbo + 168:wbo + 170].bitcast(f32)
                nc.scalar.activation(z2[:], p2[:], AF.Relu, bias=b2ap, scale=1.0)

                # L3: bias broadcast then per-(pair, block) products
                p3 = pp3.tile([128, 80], f32, tag="p3")
                nc.tensor.matmul(
                    p3[:, 0:80], ones_col[:], xw[0:1, wo + 512:wo + 592],
                    start=True, stop=False)
                for i, (p, blk) in enumerate([(p, blk) for p in range(QP)
                                              for blk in range(2)]):
                    nc.tensor.matmul(
                        p3[:, blk * 40 + p * 10: blk * 40 + p * 10 + 10],
                        z2[:, p * ST + blk * 128: p * ST + blk * 128 + 128],
                        wb[:, wbo + 128 + p * 10: wbo + 128 + p * 10 + 10],
                        start=False, stop=(i == QP * 2 - 1))
                nc.scalar.activation(exps[:, u * 80:(u + 1) * 80], p3[:], AF.Exp)

            # softmax tail over both supertiles at once
            den = spool.tile([128, 32], f32, tag="den")
            nc.vector.reduce_sum(
                den[:], exps[:].rearrange("p (g o) -> p g o", o=5),
                axis=mybir.AxisListType.X)
            rden = spool.tile([128, 32], f32, tag="rden")
            nc.vector.reciprocal_approx_fast(rden[:], den[:])
            probs = spool.tile([128, 160], f32, tag="probs")
            nc.vector.tensor_tensor(
                probs[:].rearrange("p (g o) -> p g o", o=5),
                exps[:].rearrange("p (g o) -> p g o", o=5),
                bcast_last(rden[:], 5),
                mybir.AluOpType.mult)

            nc.gpsimd.dma_start(
                out_d.ap()[2 * m:2 * m + 2].rearrange("t p kc -> p t kc"),
                probs[:].rearrange("p (t kc) -> p t kc", t=2))

    nc.compile()
    return nc


def _get_program():
    if "nc" not in _cache:
        _cache["nc"] = _build_program()
    return _cache["nc"]


# ----------------------------------------------------------------------------
# host-side data prep
# ----------------------------------------------------------------------------

def _expert_blobs(W1, W1_a, W2, W2_a, V, V_a):
    """Per-expert wa [J,64,128] and wb [J,128,172] arrays (uint16 bf16 bits)."""
    W1c = (W1[None] + W1_a).astype(np.float32)    # [J,Q,H,O+1]
    W2c = (W2[None] + W2_a).astype(np.float32)    # [J,H,H+1]
    Vc = (V[None] + V_a).astype(np.float32)       # [J,Q,O,H+1]

    wa = np.zeros((J, 12, 592), np.float32)
    for q in range(Q):
        p, s = q // 2, q % 2
        # [J, d, h] <- W1c[:, q] is [J, h, d]
        wa[:, s * 6:s * 6 + 6, p * 128 + s * 64: p * 128 + (s + 1) * 64] = \
            W1c[:, q].transpose(0, 2, 1)
        for blk in range(2):
            wa[:, 0, 512 + blk * 40 + p * 10 + s * 5:
               512 + blk * 40 + p * 10 + s * 5 + 5] = Vc[:, q, :, 0]
    wa16 = wa.astype(_bf16).view(np.uint16)   # goes into xw cols QP*ST..XW

    wb = np.zeros((J, 128, 168), np.float32)
    w2w = W2c[:, :, 1:].transpose(0, 2, 1)        # [J, i, h2]
    for s in range(2):
        wb[:, s * 64:(s + 1) * 64, s * 64:(s + 1) * 64] = w2w
    for q in range(Q):
        p, s = q // 2, q % 2
        # [J, h2, o] <- Vc[:, q, :, 1:] is [J, o, h2]
        wb[:, s * 64:(s + 1) * 64, 128 + p * 10 + s * 5: 128 + p * 10 + s * 5 + 5] = \
            Vc[:, q, :, 1:].transpose(0, 2, 1)
    wb16 = np.zeros((J, 128, 172), np.uint16)
    wb16[:, :, :168] = wb.astype(_bf16).view(np.uint16)
    b2 = np.concatenate([W2c[:, :, 0], W2c[:, :, 0]], axis=1)  # [J, 128]
    wb16[:, :, 168:170] = b2.astype(np.float32).view(np.uint16).reshape(J, 128, 2)
    return wa16, wb16


def _plan(judge_ids):
    """Supertile schedule: list of (judge, sample_idx_array), core/slot map."""
    jid = np.asarray(judge_ids).astype(np.int64).ravel()
    assert jid.shape[0] == B
    order = np.argsort(jid, kind="stable")
    counts = np.bincount(jid, minlength=J)
    tiles = []
    pos = 0
    for j in range(J):
        g = order[pos:pos + counts[j]]
        pos += counts[j]
        for s in range(0, len(g), ST):
            tiles.append((j, g[s:s + ST]))
    assert len(tiles) <= N_CORES * T, f"{len(tiles)} supertiles > capacity"
    return tiles


def _prepare_inputs(x, judge_ids, W1, W1_a, W2, W2_a, V, V_a):
    x = np.ascontiguousarray(np.asarray(x, dtype=np.float32))
    wa16, wb16 = _expert_blobs(*(np.asarray(a, dtype=np.float32)
                                 for a in (W1, W1_a, W2, W2_a, V, V_a)))
    tiles = _plan(judge_ids)

    judge_mat = np.zeros((N_CORES, T), np.int64)        # expert per slot
    xg = np.zeros((N_CORES, T, ST, Q, O), np.float32)   # gathered x
    for i, (j, g) in enumerate(tiles):
        k, t = i % N_CORES, i // N_CORES
        judge_mat[k, t] = j
        xg[k, t, :len(g)] = x[g]

    xt = np.zeros((N_CORES, T, 12, QP * ST), np.float32)
    xt[:, :, 0, :] = 1.0
    xt[:, :, 6, :] = 1.0   # dummy q=7 gets ones too; its weights are zero
    for q in range(Q):
        p, s = q // 2, q % 2
        xt[:, :, s * 6 + 1:s * 6 + 6, p * ST:(p + 1) * ST] = \
            xg[:, :, :, q, :].transpose(0, 1, 3, 2)
    xt16 = xt.astype(_bf16).view(np.uint16)

    xw16 = np.empty((N_CORES, T, 12, XW), np.uint16)
    xw16[:, :, :, :QP * ST] = xt16
    in_maps = []
    for k in range(N_CORES):
        xw16[k, :, :, QP * ST:] = wa16[judge_mat[k]]
        in_maps.append({
            "xw": np.ascontiguousarray(xw16[k]).view(_bf16),
            "wb": np.ascontiguousarray(wb16[judge_mat[k]]).view(_bf16),
        })
    return in_maps, tiles


def _assemble_output(results, tiles):
    out = np.empty((B, Q, O), np.float32)
    for i, (_, g) in enumerate(tiles):
        k, t = i % N_CORES, i // N_CORES
        blob = results[k]["out"][t].reshape(128, 2, 40)
        rows = blob.transpose(1, 0, 2).reshape(ST, 40)[:len(g), :35]
        out[g] = rows.reshape(len(g), Q, O)
    return out


# ----------------------------------------------------------------------------
# entry point
# ----------------------------------------------------------------------------

def kernel(x, judge_ids, W1, W1_a, W2, W2_a, V, V_a):
    from concourse import bass_utils
    nc = _get_program()
    in_maps, tiles = _prepare_inputs(x, judge_ids, W1, W1_a, W2, W2_a, V, V_a)
    res = bass_utils.run_bass_kernel_spmd(
        nc, in_maps, core_ids=list(range(N_CORES)), trace=False)
    return _assemble_output(res.results, tiles)


# expose for test harness reuse
def run_with_results(x, judge_ids, W1, W1_a, W2, W2_a, V, V_a, trace=False,
                     **kwargs):
    from concourse import bass_utils
    nc = _get_program()
    in_maps, tiles = _prepare_inputs(x, judge_ids, W1, W1_a, W2, W2_a, V, V_a)
    res = bass_utils.run_bass_kernel_spmd(
        nc, in_maps, core_ids=list(range(N_CORES)), trace=trace, **kwargs)
    return _assemble_output(res.results, tiles), res
